# revision 51
# baseline (speedup 1.0000x reference)
"""Sliding-window causal self-attention with RoPE on 8 Trainium2 NeuronCores.

Problem: B=2, S=2048, D=1024, H=16, HD=64, WINDOW=256, fp32.
Sharding: 2 (batch) x 4 (head-groups of 4 heads). Each core computes its
head-group's QKV projections, RoPE, windowed attention, and a partial output
projection (y_g @ Wo_g.T); the host sums the 4 partials per batch.

bf16 matmul paths everywhere (error budget 2e-2 >> bf16 noise ~4e-3) and a
software-pipelined emission schedule that keeps the PE array densely busy
(the HAM clock gate needs ~3.4us of continuous PE activity to lift the PE
clock 1.2 -> 2.4GHz, and re-throttles across idle gaps):

  chunk0 | sc0-1 | chunk1 | sc2-5 + AV qq0 + Wo st0-1 | chunk2 | sc6-9 +
  AV qq1 + Wo st2-5 | chunk3 | sc10-15 + AV qq2-3 + Wo st6-15
  (scores / AV / Wo interleaved finely so the PE always has an independent
  instruction while exp / normalize / evac chains drain on other engines)

  - x streamed in 4 s-chunks of 512 tokens, host-swizzled so every DMA row
    is one contiguous segment; all input DMAs issued upfront in need-order
    (each dma_start fans out over all 16 SDMA queues, ~300GB/s aggregate).
  - per chunk: kt-outer q/k sweep (N=512 matmuls, 4 PSUM banks), RoPE evac
    (pt2 rot matmul + DVE muls; the cos-mul runs on GpSimd from a bf16
    copy), then a v sweep with positions-on-partitions (N=256).
  - v_aug per (kb, head) = [64 ones-cols | 64 v-cols]: AV yields the softmax
    denominator replicated on PSUM rows 0:64 (base partition 0, readable by
    the custom-DVE reciprocal_approx_fast) and y on rows 64:128, so
    normalization is a 64-partition reciprocal + one DVE multiply — no
    partition broadcast, no single-partition ops.
  - window mask applied post-exp as a 0/1 bf16 multiply on the two edge
    col-subtiles of each attn tile (off the PSUM critical path, 16-bit DVE
    rate), alternating vector/gpsimd.
  - one shared PSUM tag (4 banks) serves q/k accs, rot, v accs, AV accs and
    Wo accs; + 4 score banks = 8.
  - out is stored bf16 per half-stile (host accumulates partials in f32).
"""
import sys

for _p in ("/opt/trn_rl_repo", "/root/.axon_site/_ro/trn_rl_repo"):
    if _p not in sys.path:
        sys.path.append(_p)

import numpy as np
import ml_dtypes
import concourse.bacc as bacc
import concourse.mybir as mybir
from concourse.tile import TileContext
from concourse.bass_utils import run_bass_kernel_spmd

F32 = mybir.dt.float32
BF16 = mybir.dt.bfloat16
AF = mybir.ActivationFunctionType
BF = ml_dtypes.bfloat16

B, S, D = 2, 2048, 1024
H, HD = 16, 64
WINDOW = 256
THETA = 10000.0
SCALING = 1.0

HG = 4                      # head-groups (cores per batch)
HPG = H // HG               # heads per group = 4
GD = HPG * HD               # group out width = 256
NKB = S // 128              # 16 key blocks
NSC = 4                     # 512-token s-chunks
KT = D // 128               # 8 contraction chunks
SCALE = 1.0 / float(np.sqrt(HD))
MASKVAL = -240.0

_CACHE = {}
DEBUG = False


def _build():
    nc = bacc.Bacc(target_bir_lowering=False, trn_type="TRN2")

    # x swizzled [128, (schunk, kt, 512)]: row p = x[s*512+j, kt*128+p]
    xs = nc.dram_tensor("xs", [128, NSC * KT * 512], BF16, kind="ExternalInput")
    wq = nc.dram_tensor("wq", [128, KT * GD], BF16, kind="ExternalInput")
    wk = nc.dram_tensor("wk", [128, KT * GD], BF16, kind="ExternalInput")
    wv = nc.dram_tensor("wv", [128, KT * GD], BF16, kind="ExternalInput")
    wo = nc.dram_tensor("wo", [128, (GD // 128) * D], BF16, kind="ExternalInput")
    cosb = nc.dram_tensor("cosb", [128, S], BF16, kind="ExternalInput")
    sinb = nc.dram_tensor("sinb", [128, S], BF16, kind="ExternalInput")
    pt2 = nc.dram_tensor("pt2", [128, 128], BF16, kind="ExternalInput")
    mask = nc.dram_tensor("mask", [128, 384], BF16, kind="ExternalInput")
    out = nc.dram_tensor("out", [S, D], BF16, kind="ExternalOutput")
    if DEBUG:
        d_qf = nc.dram_tensor("d_qf", [128, S], BF16, kind="ExternalOutput")
        d_kf = nc.dram_tensor("d_kf", [128, S], BF16, kind="ExternalOutput")
        d_v = nc.dram_tensor("d_v", [128, NKB * HPG * 128], BF16,
                             kind="ExternalOutput")
        d_attn = nc.dram_tensor("d_attn", [128, NKB * 384], BF16,
                                kind="ExternalOutput")
        d_yT = nc.dram_tensor("d_yT", [128, S], BF16, kind="ExternalOutput")
        d_rbs = nc.dram_tensor("d_rbs", [64, 512], F32, kind="ExternalOutput")

    with TileContext(nc) as tc:
        with tc.tile_pool(name="const", bufs=1) as cpool, \
             tc.tile_pool(name="persist", bufs=1) as ppool:
            wq_sb = cpool.tile([128, KT, GD], BF16)
            wk_sb = cpool.tile([128, KT, GD], BF16)
            wv_sb = cpool.tile([128, KT, GD], BF16)
            wo_sb = cpool.tile([128, GD // 128, D], BF16)
            cosb_sb = cpool.tile([128, S], BF16)
            sin_sb = cpool.tile([128, S], BF16)
            pt2_sb = cpool.tile([128, 128], BF16)
            mask_sb = cpool.tile([128, 384], BF16)

            # Every dma_start's descriptors fan out over all 16 SDMA queues
            # (aggregate ~300GB/s); what matters is global FIFO priority.
            # Issue strictly in need-order, alternating rings.
            xs_sb = [ppool.tile([128, KT, 512], BF16, name=f"xs{s}")
                     for s in range(NSC)]

            def xdma(s):
                tv = xs_sb[s][:].rearrange("p a b -> p (a b)")
                c0 = s * KT * 512
                nc.sync.dma_start(tv[:], xs.ap()[:, c0:c0 + KT * 512])

            xv0 = xs_sb[0][:].rearrange("p a b -> p (a b)")
            wqv = wq_sb[:].rearrange("p a b -> p (a b)")
            wkv = wk_sb[:].rearrange("p a b -> p (a b)")
            wh = KT // 2 * GD
            nc.scalar.dma_start(wqv[:, 0:wh], wq.ap()[:, 0:wh])
            nc.scalar.dma_start(wkv[:, 0:wh], wk.ap()[:, 0:wh])
            nc.sync.dma_start(xv0[:, 0:KT * 256], xs.ap()[:, 0:KT * 256])
            nc.scalar.dma_start(wqv[:, wh:2 * wh], wq.ap()[:, wh:2 * wh])
            nc.scalar.dma_start(wkv[:, wh:2 * wh], wk.ap()[:, wh:2 * wh])
            nc.sync.dma_start(xv0[:, KT * 256:KT * 512],
                              xs.ap()[:, KT * 256:KT * 512])
            nc.scalar.dma_start(pt2_sb[:], pt2[:])
            nc.scalar.dma_start(cosb_sb[:], cosb[:])
            nc.gpsimd.dma_start(sin_sb[:], sinb[:])
            nc.scalar.dma_start(wv_sb[:].rearrange("p a b -> p (a b)"), wv.ap())
            xdma(1)
            nc.gpsimd.dma_start(mask_sb[:], mask[:])
            xdma(2)
            nc.scalar.dma_start(wo_sb[:].rearrange("p a b -> p (a b)"), wo.ap())
            xdma(3)

            qf = [ppool.tile([128, S], BF16, name=f"qf{t}") for t in range(2)]
            kf = [ppool.tile([128, S], BF16, name=f"kf{t}") for t in range(2)]
            yT = [ppool.tile([128, S], BF16, name=f"yT{t}") for t in range(2)]
            # v_aug per (kb, h): [64 ones | 64 v] -> AV acc rows 0:64 hold the
            # softmax denominator (PSUM base 0, readable by the custom-DVE
            # reciprocal), rows 64:128 hold y
            v_sb = ppool.tile([128, NKB * HPG * 128], BF16)
            ones_view = v_sb[:].rearrange("p (k c) -> p k c", c=128)[:, :, 0:64]
            nc.gpsimd.memset(ones_view, 1.0)
            attns = [ppool.tile([128, NKB * 384], BF16, name=f"attn{h}")
                     for h in range(4)]

            with tc.tile_pool(name="sbtmp", bufs=4) as rawp, \
                 tc.tile_pool(name="ropet", bufs=4) as tp, \
                 tc.tile_pool(name="rbsp", bufs=6) as rbsp, \
                 tc.tile_pool(name="otp", bufs=4) as otp, \
                 tc.tile_pool(name="accps", bufs=4, space="PSUM") as accps, \
                 tc.tile_pool(name="scps", bufs=4, space="PSUM") as scps:
                rotps = accps

                wsel = [(wq_sb, 0, qf[0]), (wq_sb, 128, qf[1]),
                        (wk_sb, 0, kf[0]), (wk_sb, 128, kf[1])]



                def chunk(s):
                    s0 = s * 512
                    accs = [accps.tile([128, 512], F32, tag="acc",
                                       name=f"acc{s}_{t}") for t in range(4)]

                    def sweep(ts_, kts):
                        for kt in kts:
                            st, sp = (kt == 0), (kt == KT - 1)
                            for t in ts_:
                                w_t, off, _ = wsel[t]
                                nc.tensor.matmul(accs[t][:],
                                                 w_t[:, kt, off:off + 128],
                                                 xs_sb[s][:, kt, :],
                                                 start=st, stop=sp)

                    def rope(ts_):
                        for t in ts_:
                            dstf = wsel[t][2]
                            raw = rawp.tile([128, 512], BF16, tag="raw")
                            # alternate engine so acc slots free in parallel
                            if t % 2 == 0:
                                nc.scalar.copy(raw[:], accs[t][:])
                            else:
                                nc.vector.tensor_copy(raw[:], accs[t][:])
                            rot = rotps.tile([128, 512], F32, tag="acc",
                                             name=f"rot{s}_{t}")
                            nc.tensor.matmul(rot[:], pt2_sb[:], raw[:],
                                             start=True, stop=True)
                            t2 = tp.tile([128, 512], BF16, tag="t2")
                            nc.gpsimd.tensor_mul(t2[:], raw[:],
                                                 cosb_sb[:, s0:s0 + 512])
                            t1 = tp.tile([128, 512], BF16, tag="t1")
                            nc.vector.tensor_mul(t1[:], rot[:],
                                                 sin_sb[:, s0:s0 + 512])
                            nc.vector.tensor_add(dstf[:, s0:s0 + 512],
                                                 t1[:], t2[:])

                    # q's rope evac overlaps k's sweep; k's rope overlaps the
                    # v sweep. chunk 0 additionally ordered to DMA arrival.
                    if s == 0:
                        sweep((0, 1), range(0, 4))
                        sweep((2, 3), range(0, 4))
                        sweep((0, 1), range(4, 8))
                        rope((0, 1))
                        sweep((2, 3), range(4, 8))
                        rope((2, 3))
                    else:
                        sweep((0, 1), range(KT))
                        rope((0, 1))
                        sweep((2, 3), range(KT))
                        rope((2, 3))
                    vaccs = [accps.tile([128, 512], F32, tag="acc",
                                        name=f"vacc{s}_{j}") for j in range(2)]
                    for kt in range(KT):
                        st, sp = (kt == 0), (kt == KT - 1)
                        for j in range(2):
                            for jj in range(2):
                                pb = 2 * j + jj
                                nc.tensor.matmul(
                                    vaccs[j][:, jj * 256:(jj + 1) * 256],
                                    xs_sb[s][:, kt, pb * 128:(pb + 1) * 128],
                                    wv_sb[:, kt, 0:256],
                                    start=(st and jj == 0), stop=sp)
                    for j in range(2):
                        for jj in range(2):
                            kb = 4 * s + 2 * j + jj
                            dstv = v_sb[:, kb * HPG * 128:(kb + 1) * HPG * 128] \
                                .rearrange("p (h c) -> p h c", c=128)[:, :, 64:128]
                            srcv = vaccs[j][:, jj * 256:(jj + 1) * 256] \
                                .rearrange("p (h c) -> p h c", c=64)
                            if j == 0:
                                nc.scalar.copy(dstv, srcv)
                            else:
                                nc.vector.tensor_copy(dstv, srcv)

                def scores(kb):
                    # raw exp (no pre-mask); window mask applied after as a
                    # 0/1 bf16 multiply on the two edge col-subtiles (off the
                    # PSUM critical path, at 2x DVE 16-bit rate)
                    q0 = kb * 128
                    n = min(384, S - q0)
                    for th in range(2):
                        for i in range(2):
                            ph = 64 * i
                            h = 2 * th + i
                            sc = scps.tile([128, 384], F32, tag="sc",
                                           name=f"sc{th}_{kb}_{i}")
                            nc.tensor.matmul(sc[:, 0:n],
                                             kf[th][ph:ph + 64, q0:q0 + 128],
                                             qf[th][ph:ph + 64, q0:q0 + n],
                                             start=True, stop=True)
                            nc.scalar.activation(
                                attns[h][:, kb * 384:kb * 384 + n],
                                sc[:, 0:n], AF.Exp, scale=SCALE)
                            eng = nc.vector if h % 2 == 0 else nc.gpsimd
                            if n == 384:
                                av_ = attns[h][:, kb * 384:(kb + 1) * 384] \
                                    .rearrange("p (g c) -> p g c", g=3)[:, 0::2, :]
                                mv_ = mask_sb[:].rearrange(
                                    "p (g c) -> p g c", g=3)[:, 0::2, :]
                                eng.tensor_mul(av_, av_, mv_)
                            else:
                                sl = attns[h][:, kb * 384:kb * 384 + 128]
                                eng.tensor_mul(sl, sl, mask_sb[:, 0:128])

                def av_combo(qq, th, i, pool=None):
                    pool = pool or accps
                    h = 2 * th + i
                    attn_h = attns[h]
                    acc = pool.tile([128, 512], F32,
                                    tag="acc" if pool is accps else "sc",
                                    name=f"av{h}_{qq}")
                    first = True
                    for j2 in range(2):          # qb pair (2m, 2m+1)
                        m = 2 * qq + j2
                        qb0 = 2 * m
                        mms = []
                        if m >= 1:
                            mms.append((qb0 - 2, 0, 2 * 128, 128))
                            mms.append((qb0 - 1, 0, 128, 256))
                            mms.append((qb0, 0, 0, 256))
                        else:
                            mms.append((qb0, 0, 0, 256))
                        mms.append((qb0 + 1, 128, 0, 128))
                        for ii, (kb, jo, ao, w) in enumerate(mms):
                            wdt = min(w, S - kb * 128 - ao)
                            vcol = (kb * HPG + h) * 128
                            nc.tensor.matmul(
                                acc[:, j2 * 256 + jo:j2 * 256 + jo + wdt],
                                v_sb[:, vcol:vcol + 128],
                                attn_h[:, kb * 384 + ao:kb * 384 + ao + wdt],
                                start=first,
                                stop=(j2 == 1 and ii == len(mms) - 1))
                            first = False
                    rbs = rbsp.tile([64, 512], F32, tag="rbs")
                    nc.vector.reciprocal_approx_fast(
                        out=rbs[:], in_=acc[0:64, :])
                    nc.vector.tensor_mul(
                        yT[th][64 * i:64 * i + 64, qq * 512:(qq + 1) * 512],
                        acc[64:128, :], rbs[:])
                    if DEBUG and h == 0 and qq == 0:
                        nc.sync.dma_start(d_rbs[:], rbs[:])

                def wo_tile(stile):
                    r0 = stile * 128
                    ot = otp.tile([128, D], BF16, tag="ot")
                    for dc in range(2):
                        oacc = accps.tile([128, 512], F32, tag="acc",
                                          name=f"oacc{stile}_{dc}")
                        for ct in range(2):
                            nc.tensor.matmul(
                                oacc[:], yT[ct][:, r0:r0 + 128],
                                wo_sb[:, ct, dc * 512:(dc + 1) * 512],
                                start=(ct == 0), stop=(ct == 1))
                        if dc == 0:
                            nc.scalar.copy(ot[:, 0:512], oacc[:])
                        else:
                            nc.vector.tensor_copy(ot[:, 512:1024], oacc[:])
                        nc.sync.dma_start(
                            out.ap()[r0:r0 + 128, dc * 512:(dc + 1) * 512],
                            ot[:, dc * 512:(dc + 1) * 512])

                # pipelined schedule: every AV combo is followed by an
                # independent scores/Wo item so the PE has ready work while
                # the combo's recip+mul chain drains on the vector engine;
                # Wo tiles of window qq serve as fillers in window qq+1
                chunk(0)
                scores(0); scores(1)
                chunk(1)
                scores(2); scores(3)
                av_combo(0, 0, 0); scores(4)
                av_combo(0, 0, 1); scores(5)
                av_combo(0, 1, 0)
                chunk(2)
                av_combo(0, 1, 1)
                scores(6); wo_tile(0)
                scores(7); wo_tile(1)
                av_combo(1, 0, 0); scores(8)
                av_combo(1, 0, 1); wo_tile(2)
                av_combo(1, 1, 0); scores(9)
                av_combo(1, 1, 1); wo_tile(3)
                chunk(3)
                scores(10); wo_tile(4)
                scores(11); wo_tile(5)
                av_combo(2, 0, 0); scores(12)
                av_combo(2, 0, 1); wo_tile(6)
                av_combo(2, 1, 0); scores(13)
                av_combo(2, 1, 1); wo_tile(7)
                scores(14); scores(15)
                av_combo(3, 0, 0, scps); wo_tile(8)
                av_combo(3, 0, 1, scps); wo_tile(9)
                av_combo(3, 1, 0, scps); wo_tile(10)
                av_combo(3, 1, 1, scps); wo_tile(11)
                wo_tile(12); wo_tile(13); wo_tile(14); wo_tile(15)

            if DEBUG:
                nc.sync.dma_start(d_qf[:], qf[0][:])
                nc.sync.dma_start(d_kf[:], kf[0][:])
                nc.sync.dma_start(d_v[:], v_sb[:])
                nc.sync.dma_start(d_attn[:], attns[0][:])
                nc.sync.dma_start(d_yT[:], yT[0][:])

    nc.finalize()
    return nc


def _rope_tables():
    inv_freq = 1.0 / (THETA ** (np.arange(0, HD, 2, dtype=np.float64) / HD))
    t = np.arange(S, dtype=np.float64) / max(SCALING, 1e-6)
    freqs = np.outer(t, inv_freq)                      # [S, HD/2]
    emb = np.concatenate((freqs, freqs), axis=-1)      # [S, HD]
    return np.cos(emb).astype(np.float32), np.sin(emb).astype(np.float32)


def _swz(w):
    # [kt*128, X] -> [128, kt*X] partition-major contiguous
    kt = w.shape[0] // 128
    return np.ascontiguousarray(
        w.reshape(kt, 128, w.shape[1]).transpose(1, 0, 2).reshape(128, -1))


def _host_prep(x, Wq, Wk, Wv, Wo):
    cos, sin = _rope_tables()
    cosT2 = np.ascontiguousarray(np.tile(cos.T, (2, 1)))     # [128, S]
    sinT2 = np.ascontiguousarray(np.tile(sin.T, (2, 1)))
    P = np.zeros((HD, HD), dtype=np.float32)
    for i in range(HD // 2):
        P[2 * i, 2 * i + 1] = -1.0
        P[2 * i + 1, 2 * i] = 1.0
    PT = P.T
    pt2 = np.zeros((128, 128), dtype=np.float32)
    pt2[0:64, 0:64] = PT
    pt2[64:128, 64:128] = PT

    # multiplicative 0/1 window mask (applied to attn post-exp)
    ii = np.arange(384)[None, :]          # query offset within window
    jj = np.arange(128)[:, None]          # key offset within block
    m = np.ones((128, 384), dtype=np.float32)
    m[:, 0:128] *= (ii[:, 0:128] >= jj)
    m[:, 256:384] *= (ii[:, 256:384] - 256 < jj)

    in_maps = []
    for c in range(8):
        b, g = c // HG, c % HG
        gsl = slice(g * GD, (g + 1) * GD)
        xT = x[b].T                                         # [D, S]
        xsw = np.ascontiguousarray(
            xT.reshape(KT, 128, NSC, 512).transpose(1, 2, 0, 3)
            .reshape(128, -1)).astype(BF)
        in_maps.append({
            "xs": xsw,
            "wq": _swz(Wq[gsl, :].T).astype(BF),
            "wk": _swz(Wk[gsl, :].T).astype(BF),
            "wv": _swz(Wv[gsl, :].T).astype(BF),
            "wo": _swz(Wo[:, gsl].T).astype(BF),
            "cosb": cosT2.astype(BF), "sinb": sinT2.astype(BF),
            "pt2": pt2.astype(BF), "mask": m.astype(BF),
        })
    return in_maps


def _run(inputs, trace=False, **kw):
    if "nc" not in _CACHE:
        _CACHE["nc"] = _build()
    in_maps = _host_prep(inputs["x"], inputs["Wq"], inputs["Wk"],
                         inputs["Wv"], inputs["Wo"])
    return run_bass_kernel_spmd(_CACHE["nc"], in_maps, list(range(8)),
                                trace=trace, **kw)


def kernel(x, Wq, Wk, Wv, Wo):
    res = _run({"x": x, "Wq": Wq, "Wk": Wk, "Wv": Wv, "Wo": Wo})
    out = np.zeros((B, S, D), dtype=np.float32)
    for c in range(8):
        out[c // HG] += np.asarray(res.results[c]["out"], dtype=np.float32)
    return out


# revision 52
# speedup vs baseline: 1.0380x; 1.0380x over previous
"""Sliding-window causal self-attention with RoPE on 8 Trainium2 NeuronCores.

Problem: B=2, S=2048, D=1024, H=16, HD=64, WINDOW=256, fp32.
Sharding: 2 (batch) x 4 (head-groups of 4 heads). Each core computes its
head-group's QKV projections, RoPE, windowed attention, and a partial output
projection (y_g @ Wo_g.T); the host sums the 4 partials per batch.

bf16 matmul paths everywhere (error budget 2e-2 >> bf16 noise ~4e-3) and a
software-pipelined emission schedule that keeps the PE array densely busy
(the HAM clock gate needs ~3.4us of continuous PE activity to lift the PE
clock 1.2 -> 2.4GHz, and re-throttles across idle gaps):

  chunk0 | sc0-1 | chunk1 | sc2-5 + AV qq0 + Wo st0-1 | chunk2 | sc6-9 +
  AV qq1 + Wo st2-5 | chunk3 | sc10-15 + AV qq2-3 + Wo st6-15
  (scores / AV / Wo interleaved finely so the PE always has an independent
  instruction while exp / normalize / evac chains drain on other engines)

  - x streamed in 4 s-chunks of 512 tokens, host-swizzled so every DMA row
    is one contiguous segment; all input DMAs issued upfront in need-order
    (each dma_start fans out over all 16 SDMA queues, ~300GB/s aggregate).
  - per chunk: kt-outer q/k sweep (N=512 matmuls, 4 PSUM banks), RoPE evac
    (pt2 rot matmul + DVE muls; the cos-mul runs on GpSimd from a bf16
    copy), then a v sweep with positions-on-partitions (N=256).
  - v_aug per (kb, head) = [64 ones-cols | 64 v-cols]: AV yields the softmax
    denominator replicated on PSUM rows 0:64 (base partition 0, readable by
    the custom-DVE reciprocal_approx_fast) and y on rows 64:128, so
    normalization is a 64-partition reciprocal + one DVE multiply — no
    partition broadcast, no single-partition ops.
  - window mask applied post-exp as a 0/1 bf16 multiply on the two edge
    col-subtiles of each attn tile (off the PSUM critical path, 16-bit DVE
    rate), alternating vector/gpsimd.
  - one shared PSUM tag (4 banks) serves q/k accs, rot, v accs, AV accs and
    Wo accs; + 4 score banks = 8.
  - out is stored bf16 per half-stile (host accumulates partials in f32).
"""
import sys

for _p in ("/opt/trn_rl_repo", "/root/.axon_site/_ro/trn_rl_repo"):
    if _p not in sys.path:
        sys.path.append(_p)

import numpy as np
import ml_dtypes
import concourse.bacc as bacc
import concourse.mybir as mybir
from concourse.tile import TileContext
from concourse.bass_utils import run_bass_kernel_spmd

F32 = mybir.dt.float32
BF16 = mybir.dt.bfloat16
AF = mybir.ActivationFunctionType
BF = ml_dtypes.bfloat16

B, S, D = 2, 2048, 1024
H, HD = 16, 64
WINDOW = 256
THETA = 10000.0
SCALING = 1.0

HG = 4                      # head-groups (cores per batch)
HPG = H // HG               # heads per group = 4
GD = HPG * HD               # group out width = 256
NKB = S // 128              # 16 key blocks
NSC = 4                     # 512-token s-chunks
KT = D // 128               # 8 contraction chunks
SCALE = 1.0 / float(np.sqrt(HD))
MASKVAL = -240.0

_CACHE = {}
DEBUG = False


def _build():
    nc = bacc.Bacc(target_bir_lowering=False, trn_type="TRN2")

    # x swizzled [128, (schunk, kt, 512)]: row p = x[s*512+j, kt*128+p]
    xs = nc.dram_tensor("xs", [128, NSC * KT * 512], BF16, kind="ExternalInput")
    wq = nc.dram_tensor("wq", [128, KT * GD], BF16, kind="ExternalInput")
    wk = nc.dram_tensor("wk", [128, KT * GD], BF16, kind="ExternalInput")
    wv = nc.dram_tensor("wv", [128, KT * GD], BF16, kind="ExternalInput")
    wo = nc.dram_tensor("wo", [128, (GD // 128) * D], BF16, kind="ExternalInput")
    cosb = nc.dram_tensor("cosb", [128, S], BF16, kind="ExternalInput")
    sinb = nc.dram_tensor("sinb", [128, S], BF16, kind="ExternalInput")
    pt2 = nc.dram_tensor("pt2", [128, 128], BF16, kind="ExternalInput")
    mask = nc.dram_tensor("mask", [128, 384], BF16, kind="ExternalInput")
    out = nc.dram_tensor("out", [S, D], BF16, kind="ExternalOutput")
    if DEBUG:
        d_qf = nc.dram_tensor("d_qf", [128, S], BF16, kind="ExternalOutput")
        d_kf = nc.dram_tensor("d_kf", [128, S], BF16, kind="ExternalOutput")
        d_v = nc.dram_tensor("d_v", [128, NKB * HPG * 128], BF16,
                             kind="ExternalOutput")
        d_attn = nc.dram_tensor("d_attn", [128, NKB * 384], BF16,
                                kind="ExternalOutput")
        d_yT = nc.dram_tensor("d_yT", [128, S], BF16, kind="ExternalOutput")
        d_rbs = nc.dram_tensor("d_rbs", [64, 512], F32, kind="ExternalOutput")

    with TileContext(nc) as tc:
        with tc.tile_pool(name="const", bufs=1) as cpool, \
             tc.tile_pool(name="persist", bufs=1) as ppool:
            wq_sb = cpool.tile([128, KT, GD], BF16)
            wk_sb = cpool.tile([128, KT, GD], BF16)
            wv_sb = cpool.tile([128, KT, GD], BF16)
            wo_sb = cpool.tile([128, GD // 128, D], BF16)
            cosb_sb = cpool.tile([128, S], BF16)
            sin_sb = cpool.tile([128, S], BF16)
            pt2_sb = cpool.tile([128, 128], BF16)
            mask_sb = cpool.tile([128, 384], BF16)

            # Every dma_start's descriptors fan out over all 16 SDMA queues
            # (aggregate ~300GB/s); what matters is global FIFO priority.
            # Issue strictly in need-order, alternating rings.
            xs_sb = [ppool.tile([128, KT, 512], BF16, name=f"xs{s}")
                     for s in range(NSC)]

            def xdma(s):
                tv = xs_sb[s][:].rearrange("p a b -> p (a b)")
                c0 = s * KT * 512
                nc.sync.dma_start(tv[:], xs.ap()[:, c0:c0 + KT * 512])

            xv0 = xs_sb[0][:].rearrange("p a b -> p (a b)")
            wqv = wq_sb[:].rearrange("p a b -> p (a b)")
            wkv = wk_sb[:].rearrange("p a b -> p (a b)")
            wh = KT // 2 * GD
            nc.scalar.dma_start(wqv[:, 0:wh], wq.ap()[:, 0:wh])
            nc.scalar.dma_start(wkv[:, 0:wh], wk.ap()[:, 0:wh])
            nc.sync.dma_start(xv0[:, 0:KT * 256], xs.ap()[:, 0:KT * 256])
            nc.scalar.dma_start(wqv[:, wh:2 * wh], wq.ap()[:, wh:2 * wh])
            nc.scalar.dma_start(wkv[:, wh:2 * wh], wk.ap()[:, wh:2 * wh])
            nc.sync.dma_start(xv0[:, KT * 256:KT * 512],
                              xs.ap()[:, KT * 256:KT * 512])
            nc.scalar.dma_start(pt2_sb[:], pt2[:])
            nc.scalar.dma_start(cosb_sb[:], cosb[:])
            nc.gpsimd.dma_start(sin_sb[:], sinb[:])
            nc.scalar.dma_start(wv_sb[:].rearrange("p a b -> p (a b)"), wv.ap())
            xdma(1)
            nc.gpsimd.dma_start(mask_sb[:], mask[:])
            xdma(2)
            nc.scalar.dma_start(wo_sb[:].rearrange("p a b -> p (a b)"), wo.ap())
            xdma(3)

            qf = [ppool.tile([128, S], BF16, name=f"qf{t}") for t in range(2)]
            kf = [ppool.tile([128, S], BF16, name=f"kf{t}") for t in range(2)]
            yT = [ppool.tile([128, S], BF16, name=f"yT{t}") for t in range(2)]
            # v_aug per (kb, h): [64 ones | 64 v] -> AV acc rows 0:64 hold the
            # softmax denominator (PSUM base 0, readable by the custom-DVE
            # reciprocal), rows 64:128 hold y
            v_sb = ppool.tile([128, NKB * HPG * 128], BF16)
            ones_view = v_sb[:].rearrange("p (k c) -> p k c", c=128)[:, :, 0:64]
            nc.gpsimd.memset(ones_view, 1.0)
            attns = [ppool.tile([128, NKB * 384], BF16, name=f"attn{h}")
                     for h in range(4)]

            with tc.tile_pool(name="sbtmp", bufs=4) as rawp, \
                 tc.tile_pool(name="ropet", bufs=4) as tp, \
                 tc.tile_pool(name="rbsp", bufs=6) as rbsp, \
                 tc.tile_pool(name="otp", bufs=4) as otp, \
                 tc.tile_pool(name="accps", bufs=4, space="PSUM") as accps, \
                 tc.tile_pool(name="scps", bufs=4, space="PSUM") as scps:
                rotps = accps

                wsel = [(wq_sb, 0, qf[0]), (wq_sb, 128, qf[1]),
                        (wk_sb, 0, kf[0]), (wk_sb, 128, kf[1])]



                def chunk(s):
                    s0 = s * 512
                    accs = [accps.tile([128, 512], F32, tag="acc",
                                       name=f"acc{s}_{t}") for t in range(4)]

                    def sweep(ts_, kts):
                        for kt in kts:
                            st, sp = (kt == 0), (kt == KT - 1)
                            for t in ts_:
                                w_t, off, _ = wsel[t]
                                nc.tensor.matmul(accs[t][:],
                                                 w_t[:, kt, off:off + 128],
                                                 xs_sb[s][:, kt, :],
                                                 start=st, stop=sp)

                    def rope(ts_):
                        for t in ts_:
                            dstf = wsel[t][2]
                            raw = rawp.tile([128, 512], BF16, tag="raw")
                            # alternate engine so acc slots free in parallel
                            if t % 2 == 0:
                                nc.scalar.copy(raw[:], accs[t][:])
                            else:
                                nc.vector.tensor_copy(raw[:], accs[t][:])
                            rot = rotps.tile([128, 512], F32, tag="acc",
                                             name=f"rot{s}_{t}")
                            nc.tensor.matmul(rot[:], pt2_sb[:], raw[:],
                                             start=True, stop=True)
                            t2 = tp.tile([128, 512], BF16, tag="t2")
                            nc.gpsimd.tensor_mul(t2[:], raw[:],
                                                 cosb_sb[:, s0:s0 + 512])
                            t1 = tp.tile([128, 512], BF16, tag="t1")
                            nc.vector.tensor_mul(t1[:], rot[:],
                                                 sin_sb[:, s0:s0 + 512])
                            nc.vector.tensor_add(dstf[:, s0:s0 + 512],
                                                 t1[:], t2[:])

                    # q's rope evac overlaps k's sweep; k's rope overlaps the
                    # v sweep. chunk 0 additionally ordered to DMA arrival.
                    if s == 0:
                        sweep((0, 1), range(0, 4))
                        sweep((2, 3), range(0, 4))
                        sweep((0, 1), range(4, 8))
                        rope((0, 1))
                        sweep((2, 3), range(4, 8))
                        rope((2, 3))
                    else:
                        sweep((0, 1), range(KT))
                        rope((0, 1))
                        sweep((2, 3), range(KT))
                        rope((2, 3))
                    vaccs = [accps.tile([128, 512], F32, tag="acc",
                                        name=f"vacc{s}_{j}") for j in range(2)]
                    for kt in range(KT):
                        st, sp = (kt == 0), (kt == KT - 1)
                        for j in range(2):
                            for jj in range(2):
                                pb = 2 * j + jj
                                nc.tensor.matmul(
                                    vaccs[j][:, jj * 256:(jj + 1) * 256],
                                    xs_sb[s][:, kt, pb * 128:(pb + 1) * 128],
                                    wv_sb[:, kt, 0:256],
                                    start=(st and jj == 0), stop=sp)
                    for j in range(2):
                        for jj in range(2):
                            kb = 4 * s + 2 * j + jj
                            dstv = v_sb[:, kb * HPG * 128:(kb + 1) * HPG * 128] \
                                .rearrange("p (h c) -> p h c", c=128)[:, :, 64:128]
                            srcv = vaccs[j][:, jj * 256:(jj + 1) * 256] \
                                .rearrange("p (h c) -> p h c", c=64)
                            if j == 0:
                                nc.scalar.copy(dstv, srcv)
                            else:
                                nc.vector.tensor_copy(dstv, srcv)

                def scores(kb):
                    # raw exp (no pre-mask); window mask applied after as a
                    # 0/1 bf16 multiply on the two edge col-subtiles (off the
                    # PSUM critical path, at 2x DVE 16-bit rate)
                    q0 = kb * 128
                    n = min(384, S - q0)
                    for th in range(2):
                        for i in range(2):
                            ph = 64 * i
                            h = 2 * th + i
                            sc = scps.tile([128, 384], F32, tag="sc",
                                           name=f"sc{th}_{kb}_{i}")
                            nc.tensor.matmul(sc[:, 0:n],
                                             kf[th][ph:ph + 64, q0:q0 + 128],
                                             qf[th][ph:ph + 64, q0:q0 + n],
                                             start=True, stop=True)
                            nc.scalar.activation(
                                attns[h][:, kb * 384:kb * 384 + n],
                                sc[:, 0:n], AF.Exp, scale=SCALE)
                            eng = nc.vector if h % 2 == 0 else nc.gpsimd
                            if n == 384:
                                av_ = attns[h][:, kb * 384:(kb + 1) * 384] \
                                    .rearrange("p (g c) -> p g c", g=3)[:, 0::2, :]
                                mv_ = mask_sb[:].rearrange(
                                    "p (g c) -> p g c", g=3)[:, 0::2, :]
                                eng.tensor_mul(av_, av_, mv_)
                            else:
                                sl = attns[h][:, kb * 384:kb * 384 + 128]
                                eng.tensor_mul(sl, sl, mask_sb[:, 0:128])

                def av_combo(qq, th, i):
                    h = 2 * th + i
                    attn_h = attns[h]
                    acc = accps.tile([128, 512], F32, tag="acc",
                                     name=f"av{h}_{qq}")
                    first = True
                    for j2 in range(2):          # qb pair (2m, 2m+1)
                        m = 2 * qq + j2
                        qb0 = 2 * m
                        mms = []
                        if m >= 1:
                            mms.append((qb0 - 2, 0, 2 * 128, 128))
                            mms.append((qb0 - 1, 0, 128, 256))
                            mms.append((qb0, 0, 0, 256))
                        else:
                            mms.append((qb0, 0, 0, 256))
                        mms.append((qb0 + 1, 128, 0, 128))
                        for ii, (kb, jo, ao, w) in enumerate(mms):
                            wdt = min(w, S - kb * 128 - ao)
                            vcol = (kb * HPG + h) * 128
                            nc.tensor.matmul(
                                acc[:, j2 * 256 + jo:j2 * 256 + jo + wdt],
                                v_sb[:, vcol:vcol + 128],
                                attn_h[:, kb * 384 + ao:kb * 384 + ao + wdt],
                                start=first,
                                stop=(j2 == 1 and ii == len(mms) - 1))
                            first = False
                    rbs = rbsp.tile([64, 512], F32, tag="rbs")
                    nc.vector.reciprocal_approx_fast(
                        out=rbs[:], in_=acc[0:64, :])
                    nc.vector.tensor_mul(
                        yT[th][64 * i:64 * i + 64, qq * 512:(qq + 1) * 512],
                        acc[64:128, :], rbs[:])
                    if DEBUG and h == 0 and qq == 0:
                        nc.sync.dma_start(d_rbs[:], rbs[:])

                def wo_tile(stile):
                    r0 = stile * 128
                    ot = otp.tile([128, D], BF16, tag="ot")
                    for dc in range(2):
                        oacc = accps.tile([128, 512], F32, tag="acc",
                                          name=f"oacc{stile}_{dc}")
                        for ct in range(2):
                            nc.tensor.matmul(
                                oacc[:], yT[ct][:, r0:r0 + 128],
                                wo_sb[:, ct, dc * 512:(dc + 1) * 512],
                                start=(ct == 0), stop=(ct == 1))
                        if dc == 0:
                            nc.scalar.copy(ot[:, 0:512], oacc[:])
                        else:
                            nc.vector.tensor_copy(ot[:, 512:1024], oacc[:])
                        nc.sync.dma_start(
                            out.ap()[r0:r0 + 128, dc * 512:(dc + 1) * 512],
                            ot[:, dc * 512:(dc + 1) * 512])

                # pipelined schedule: every AV combo is followed by an
                # independent scores/Wo item so the PE has ready work while
                # the combo's recip+mul chain drains on the vector engine;
                # Wo tiles of window qq serve as fillers in window qq+1
                chunk(0)
                scores(0); scores(1)
                chunk(1)
                scores(2); scores(3)
                av_combo(0, 0, 0); scores(4)
                av_combo(0, 0, 1); scores(5)
                av_combo(0, 1, 0)
                chunk(2)
                av_combo(0, 1, 1)
                scores(6); wo_tile(0)
                scores(7); wo_tile(1)
                av_combo(1, 0, 0); scores(8)
                av_combo(1, 0, 1); wo_tile(2)
                av_combo(1, 1, 0); scores(9)
                av_combo(1, 1, 1); wo_tile(3)
                chunk(3)
                scores(10); wo_tile(4)
                scores(11); wo_tile(5)
                av_combo(2, 0, 0); scores(12)
                av_combo(2, 0, 1); wo_tile(6)
                av_combo(2, 1, 0); scores(13)
                av_combo(2, 1, 1); wo_tile(7)
                scores(14); scores(15)
                av_combo(3, 0, 0); wo_tile(8)
                av_combo(3, 0, 1); wo_tile(9)
                av_combo(3, 1, 0); wo_tile(10)
                av_combo(3, 1, 1); wo_tile(11)
                wo_tile(12); wo_tile(13); wo_tile(14); wo_tile(15)

            if DEBUG:
                nc.sync.dma_start(d_qf[:], qf[0][:])
                nc.sync.dma_start(d_kf[:], kf[0][:])
                nc.sync.dma_start(d_v[:], v_sb[:])
                nc.sync.dma_start(d_attn[:], attns[0][:])
                nc.sync.dma_start(d_yT[:], yT[0][:])

    nc.finalize()
    return nc


def _rope_tables():
    inv_freq = 1.0 / (THETA ** (np.arange(0, HD, 2, dtype=np.float64) / HD))
    t = np.arange(S, dtype=np.float64) / max(SCALING, 1e-6)
    freqs = np.outer(t, inv_freq)                      # [S, HD/2]
    emb = np.concatenate((freqs, freqs), axis=-1)      # [S, HD]
    return np.cos(emb).astype(np.float32), np.sin(emb).astype(np.float32)


def _swz(w):
    # [kt*128, X] -> [128, kt*X] partition-major contiguous
    kt = w.shape[0] // 128
    return np.ascontiguousarray(
        w.reshape(kt, 128, w.shape[1]).transpose(1, 0, 2).reshape(128, -1))


def _host_prep(x, Wq, Wk, Wv, Wo):
    cos, sin = _rope_tables()
    cosT2 = np.ascontiguousarray(np.tile(cos.T, (2, 1)))     # [128, S]
    sinT2 = np.ascontiguousarray(np.tile(sin.T, (2, 1)))
    P = np.zeros((HD, HD), dtype=np.float32)
    for i in range(HD // 2):
        P[2 * i, 2 * i + 1] = -1.0
        P[2 * i + 1, 2 * i] = 1.0
    PT = P.T
    pt2 = np.zeros((128, 128), dtype=np.float32)
    pt2[0:64, 0:64] = PT
    pt2[64:128, 64:128] = PT

    # multiplicative 0/1 window mask (applied to attn post-exp)
    ii = np.arange(384)[None, :]          # query offset within window
    jj = np.arange(128)[:, None]          # key offset within block
    m = np.ones((128, 384), dtype=np.float32)
    m[:, 0:128] *= (ii[:, 0:128] >= jj)
    m[:, 256:384] *= (ii[:, 256:384] - 256 < jj)

    in_maps = []
    for c in range(8):
        b, g = c // HG, c % HG
        gsl = slice(g * GD, (g + 1) * GD)
        xT = x[b].T                                         # [D, S]
        xsw = np.ascontiguousarray(
            xT.reshape(KT, 128, NSC, 512).transpose(1, 2, 0, 3)
            .reshape(128, -1)).astype(BF)
        in_maps.append({
            "xs": xsw,
            "wq": _swz(Wq[gsl, :].T).astype(BF),
            "wk": _swz(Wk[gsl, :].T).astype(BF),
            "wv": _swz(Wv[gsl, :].T).astype(BF),
            "wo": _swz(Wo[:, gsl].T).astype(BF),
            "cosb": cosT2.astype(BF), "sinb": sinT2.astype(BF),
            "pt2": pt2.astype(BF), "mask": m.astype(BF),
        })
    return in_maps


def _run(inputs, trace=False, **kw):
    if "nc" not in _CACHE:
        _CACHE["nc"] = _build()
    in_maps = _host_prep(inputs["x"], inputs["Wq"], inputs["Wk"],
                         inputs["Wv"], inputs["Wo"])
    return run_bass_kernel_spmd(_CACHE["nc"], in_maps, list(range(8)),
                                trace=trace, **kw)


def kernel(x, Wq, Wk, Wv, Wo):
    res = _run({"x": x, "Wq": Wq, "Wk": Wk, "Wv": Wv, "Wo": Wo})
    out = np.zeros((B, S, D), dtype=np.float32)
    for c in range(8):
        out[c // HG] += np.asarray(res.results[c]["out"], dtype=np.float32)
    return out
